# revision 14
# baseline (speedup 1.0000x reference)
"""Trainium2 Bass kernel for nn_EnhancedSNN (2-layer LIF spiking net).

Math (reference):
    current1 = x @ W1.T + b1                      # [B, HID], constant over time
    per step t in 0..31:
        v1 = 0.9*v1 + current1; s1 = (v1>=0.8); v1 = where(s1, 0, v1)
        current2 = s1 @ W2.T + b2                 # [B, OUT]
        v2 = 0.9*v2 + current2; s2 = (v2>=0.5); v2 = where(s2, 0, v2)
    spike_tensor = stack_t(s2) -> [B, T, OUT]
    output = mean_t(s2) @ Wr.T + br               # [B, 2]

Strategy:
  - Data parallel over batch: B=2048 -> 8 cores x 256 rows.
  - Device computes fc1 + both LIF recurrences + the per-step fc2 matmuls,
    returning the spike tensor slice per core. The tiny 2-unit head and the
    (exact, fp32) mean over T run on the host.
  - All matmuls run as bf16 multi-term splits at 1 cycle/row per pass on the
    PE: fc1 uses triple splits of x and W1 (6 term passes, ~2^-26 inputs,
    needed because spike thresholds amplify any current1 error), fc2 uses
    hi/lo W2 (2 passes, ~2^-18 weights).
  - Layouts: layer-1 state is [h_partition, b_free]; s1 is generated there
    and used directly as the matmul stationary operand; fc2 PSUM output is
    [b_partition, o_free], which is both the LIF-2 layout and the contiguous
    HBM layout for the spike tensor.
  - W2 hi half stays resident in SBUF; the lo half streams per step.
"""

import os

import numpy as np
import ml_dtypes

import concourse.bacc as bacc
import concourse.mybir as mybir
import concourse.tile as tile
from concourse.bass_utils import run_bass_kernel_spmd

P = 128
B, IN, HID, OUT = 2048, 1024, 4096, 1024
T = 32
NCORES = 8
BL = B // NCORES            # 256 batch rows per core
KT1 = IN // P               # 8 k-tiles for fc1
HT = HID // P               # 32 h-tiles
THR1, THR2 = 0.8, 0.5
LEAK = 0.9

F32 = mybir.dt.float32
BF16 = mybir.dt.bfloat16
ALU = mybir.AluOpType

_CACHED_NC = None


def _build():
    nc = bacc.Bacc("TRN2", target_bir_lowering=False, debug=False)

    # ---- inputs (host pre-shaped, partition-major, bf16 hi/lo pairs) ----
    xt_d = [nc.dram_tensor(f"xt{i}", [P, KT1, BL], BF16, kind="ExternalInput")
            for i in range(3)]
    w1_d = [nc.dram_tensor(f"w1{i}", [P, KT1, HID], BF16, kind="ExternalInput")
            for i in range(3)]
    w2h_d = nc.dram_tensor("w2h", [P, HT, OUT], BF16, kind="ExternalInput")
    w2l_d = nc.dram_tensor("w2l", [HT, P, OUT], BF16, kind="ExternalInput")
    b1_d = nc.dram_tensor("b1r", [P, HT], F32, kind="ExternalInput")
    b2_d = nc.dram_tensor("b2r", [P, OUT], F32, kind="ExternalInput")

    spike_d = nc.dram_tensor("spike", [BL, T, OUT], F32, kind="ExternalOutput")

    with tile.TileContext(nc) as tc:
        with (
            tc.tile_pool(name="const", bufs=1) as constp,
            tc.tile_pool(name="bigw", bufs=1) as bigw,
            tc.tile_pool(name="state", bufs=1) as state,
            tc.tile_pool(name="s1p", bufs=2) as s1p,
            tc.tile_pool(name="wch", bufs=1) as wch,
            tc.tile_pool(name="w2lo", bufs=3) as lop,
            tc.tile_pool(name="s2p", bufs=2) as s2p,
            tc.tile_pool(name="l2scr", bufs=1) as l2scr,
            tc.tile_pool(name="psum", bufs=2, space="PSUM") as psum,
        ):
            # ---- constants ----
            b1_sb = constp.tile([P, HT], F32)
            nc.sync.dma_start(b1_sb[:], b1_d[:, :])
            b2_sb = constp.tile([P, OUT], F32)
            nc.sync.dma_start(b2_sb[:], b2_d[:, :])

            # x hi/mid/lo tiles (exact to 2^-27); tiny, live through fc1 only
            xt = []
            for i in range(3):
                xt.append(constp.tile([P, KT1, BL], BF16, name=f"xt{i}_t"))
                nc.sync.dma_start(xt[i][:], xt_d[i][:, :, :])

            c1 = state.tile([P, HT, BL], F32)
            v1 = state.tile([P, HT, BL], F32)
            v2 = state.tile([P, 2, OUT], F32)
            nc.vector.memset(v1[:], 0.0)
            nc.vector.memset(v2[:], 0.0)

            def fc1_psum(i):
                return psum.tile([P, BL], F32, tag=f"q{(i // 2) % 2}{i % 2}",
                                 name=f"fc1ps{i}")

            # ---- fc1: three weight sweeps, ping-ponged through TWO 32KB
            # half-slots (h-halves) so each half's DMA overlaps the other
            # half's matmuls. Terms kept:
            #   w0*(x0+x1+x2) + w1*(x0+x1) + w2*x0   (~2^-26 inputs)
            HH = HT // 2  # 16 h-tiles per half
            for j in range(3):
                nx = 3 - j
                for half in range(2):
                    tag = f"half{half}"
                    w1jh = bigw.tile([P, KT1, HID // 2], BF16, tag=tag,
                                     name=f"w1s{j}h{half}_t")
                    h0 = half * (HID // 2)
                    for kt in range(KT1):
                        nc.sync.dma_start(
                            w1jh[:, kt], w1_d[j][:, kt, h0:h0 + HID // 2])
                    for hl in range(HH):
                        ht = half * HH + hl
                        ps = fc1_psum(ht)
                        for kt in range(KT1):
                            lw = w1jh[:, kt, hl * P:(hl + 1) * P]
                            for i in range(nx):
                                nc.tensor.matmul(
                                    ps[:], lw, xt[i][:, kt],
                                    start=(kt == 0 and i == 0),
                                    stop=(kt == KT1 - 1 and i == nx - 1))
                        if j == 0:
                            nc.vector.tensor_scalar(
                                c1[:, ht], ps[:], b1_sb[:, ht:ht + 1], None,
                                ALU.add)
                        else:
                            nc.vector.tensor_tensor(
                                c1[:, ht], c1[:, ht], ps[:], ALU.add)

            # ---- W2 hi resident, split across the two half-slots so each
            # half's load hides behind the preceding compute ----
            w2h_half = []
            for half in range(2):
                w2hh = bigw.tile([P, HH, OUT], BF16, tag=f"half{half}",
                                 name=f"w2h{half}_t")
                for kt in range(0, HH, 4):
                    nc.sync.dma_start(
                        w2hh[:, kt:kt + 4],
                        w2h_d[:, half * HH + kt:half * HH + kt + 4, :])
                w2h_half.append(w2hh)

            # ---- time loop ----
            for t in range(T):
                # layer-1 LIF, chunked, never aliasing in/out APs
                s1s = s1p.tile([P, HT, BL], BF16, tag="s1")
                CH = 4
                for ch in range(0, HT, CH):
                    sl = slice(ch, ch + CH)
                    wv = wch.tile([P, CH, BL], F32, tag="wch", name=f"w{ch}")
                    nc.vector.scalar_tensor_tensor(
                        wv[:], v1[:, sl], LEAK, c1[:, sl], ALU.mult, ALU.add)
                    nc.vector.tensor_scalar(
                        s1s[:, sl], wv[:], THR1, None, ALU.is_ge)
                    nc.vector.scalar_tensor_tensor(
                        v1[:, sl], wv[:], THR1, wv[:], ALU.is_lt, ALU.mult)

                # fc2: psum[b, o] quarters accumulate over 32 k-tiles x 2 passes
                pq = {}
                for m in range(2):
                    for n in range(2):
                        pq[(m, n)] = psum.tile([P, 512], F32, tag=f"q{m}{n}", name=f"pq{m}{n}")
                for kt in range(HT):
                    lo_t = lop.tile([P, OUT], BF16, tag="w2lo")
                    nc.sync.dma_start(lo_t[:], w2l_d[kt, :, :])
                    w2hh = w2h_half[kt // HH]
                    kh = kt % HH
                    for m in range(2):
                        lhs = s1s[:, kt, m * P:(m + 1) * P]
                        for n in range(2):
                            nc.tensor.matmul(
                                pq[(m, n)][:], lhs,
                                w2hh[:, kh, n * 512:(n + 1) * 512],
                                start=(kt == 0), stop=False)
                            nc.tensor.matmul(
                                pq[(m, n)][:], lhs,
                                lo_t[:, n * 512:(n + 1) * 512],
                                start=False, stop=(kt == HT - 1))

                # layer-2 LIF + write spikes
                for m in range(2):
                    w2s = l2scr.tile([P, OUT], F32, tag="l2w")
                    nc.vector.tensor_scalar_mul(w2s[:], v2[:, m], LEAK)
                    for n in range(2):
                        sl = slice(n * 512, (n + 1) * 512)
                        nc.vector.tensor_tensor(
                            w2s[:, sl], w2s[:, sl], pq[(m, n)][:], ALU.add)
                    nc.vector.tensor_tensor(
                        w2s[:], w2s[:], b2_sb[:], ALU.add)
                    s2t = s2p.tile([P, OUT], F32, tag="s2")
                    nc.vector.tensor_scalar(s2t[:], w2s[:], THR2, None, ALU.is_ge)
                    nc.vector.scalar_tensor_tensor(
                        v2[:, m], w2s[:], THR2, w2s[:], ALU.is_lt, ALU.mult)
                    nc.sync.dma_start(spike_d[m * P:(m + 1) * P, t, :], s2t[:])

    nc.compile()
    return nc


def _split_bf16(a):
    hi = a.astype(ml_dtypes.bfloat16)
    lo = (a - hi.astype(np.float32)).astype(ml_dtypes.bfloat16)
    return np.ascontiguousarray(hi), np.ascontiguousarray(lo)


def _split_bf16_3(a):
    h0 = a.astype(ml_dtypes.bfloat16)
    r1 = a - h0.astype(np.float32)
    h1 = r1.astype(ml_dtypes.bfloat16)
    h2 = (r1 - h1.astype(np.float32)).astype(ml_dtypes.bfloat16)
    return [np.ascontiguousarray(h) for h in (h0, h1, h2)]


def kernel(x, W1, b1, W2, b2, Wr, br, num_steps):
    x = np.asarray(x, dtype=np.float32)
    W1 = np.asarray(W1, dtype=np.float32)
    b1 = np.asarray(b1, dtype=np.float32)
    W2 = np.asarray(W2, dtype=np.float32)
    b2 = np.asarray(b2, dtype=np.float32)
    Wr = np.asarray(Wr, dtype=np.float32)
    br = np.asarray(br, dtype=np.float32)
    assert int(num_steps) == T, f"kernel compiled for T={T}, got {num_steps}"

    global _CACHED_NC
    if _CACHED_NC is None:
        _CACHED_NC = _build()
    nc = _CACHED_NC

    # ---- host-side data marshaling ----
    # x.T -> [IN, B] -> per-core [P, KT1, BL]
    xT = np.ascontiguousarray(x.T)                      # [IN, B]
    xsp = _split_bf16_3(xT)
    # W1.T -> [IN, HID] -> [P, KT1, HID]
    w1T = np.ascontiguousarray(W1.T)                    # [IN, HID]
    w1sp = [np.ascontiguousarray(w.reshape(KT1, P, HID).transpose(1, 0, 2))
            for w in _split_bf16_3(w1T)]
    # W2.T -> [HID, OUT]; hi as [P, HT, OUT], lo as [HT, P, OUT]
    w2T = np.ascontiguousarray(W2.T)                    # [HID, OUT]
    w2h, w2l = _split_bf16(w2T)
    w2h = np.ascontiguousarray(w2h.reshape(HT, P, OUT).transpose(1, 0, 2))
    w2l = np.ascontiguousarray(w2l.reshape(HT, P, OUT))
    # biases pre-broadcast
    b1r = np.ascontiguousarray(b1.reshape(HT, P).T)     # [P, HT]
    b2r = np.ascontiguousarray(np.broadcast_to(b2, (P, OUT)))

    in_maps = []
    for c in range(NCORES):
        sl = slice(c * BL, (c + 1) * BL)
        m = {"w2h": w2h, "w2l": w2l, "b1r": b1r, "b2r": b2r}
        for i in range(3):
            m[f"xt{i}"] = np.ascontiguousarray(
                xsp[i][:, sl].reshape(KT1, P, BL).transpose(1, 0, 2))
            m[f"w1{i}"] = w1sp[i]
        in_maps.append(m)

    kwargs = {}
    if os.environ.get("SNN_TRACE") == "1":
        kwargs = {"trace": True, "trace_cores": [0]}
    res = run_bass_kernel_spmd(nc, in_maps, core_ids=list(range(NCORES)), **kwargs)
    global LAST_RESULTS
    LAST_RESULTS = res

    spike_tensor = np.concatenate(
        [res.results[c]["spike"] for c in range(NCORES)], axis=0)

    # exact: sums of 0/1 over 32 steps are integers, /32 is a power of two
    spike_rate = spike_tensor.sum(axis=1, dtype=np.float32) / np.float32(T)
    output = spike_rate @ Wr.T.astype(np.float32) + br

    return output, spike_tensor
